# revision 11
# baseline (speedup 1.0000x reference)
"""4-D average pool (kernel=2, stride=2) over [2,16,32,32,32,32] f32, on 8 NeuronCores.

Strategy: data-parallel over the 32 (b,c) slices -> 4 slices per core; the
per-core input is a contiguous [4096, 1024] f32 block (rows = (slice,d1,d2),
cols = (d3,d4)).

Each load tile covers 128 output rows (partition = (o1', o2), i.e. the
pooled d1/d2 index).  The d1-pair reduction is done BY THE DMA: two SWDGE
sub-DMAs per tile, the second with accum_op=add (the SDMA CCE unit adds
inline at line rate).  The d2-pairs are adjacent input rows, so they merge
with the column dim into contiguous 8 KiB runs and land as the free dim
(e2, d3, d4).  DVE then pools e2-halves, d4-pairs, d3-pairs and scales by
1/16.  No matmul, no PSUM, no ScalarE.
"""

import sys

import numpy as np

if "/opt/trn_rl_repo" not in sys.path:
    sys.path.insert(0, "/opt/trn_rl_repo")

import concourse.bacc as bacc
import concourse.bass as bass
import concourse.tile as tile
from concourse import mybir
from concourse.bass_utils import run_bass_kernel_spmd

N_CORES = 8
SLICES_PER_CORE = 4  # 32 (b,c) slices / 8 cores
ROWS = SLICES_PER_CORE * 1024  # 4096
N_TILES = 8  # one per 128 output rows; 2 MiB of input each
LAG = 2  # accumulating sub-DMA trails the initial one by LAG tiles
F32 = mybir.dt.float32
ADD = mybir.AluOpType.add


def build_nc() -> bass.Bass:
    # Bacc (not raw Bass): its compile() splits multi-sem sync waits into
    # event-semaphore instructions (TRN2 allows one wait per instruction).
    nc = bacc.Bacc()
    x = nc.dram_tensor("x", [ROWS, 1024], F32, kind="ExternalInput")
    y = nc.dram_tensor("y", [ROWS // 4, 256], F32, kind="ExternalOutput")

    # x row = 1024*s + 512*h + 64*o1p + 32*e1 + 2*o2 + e2 ; output row =
    # 256*s + 128*h + 16*o1p + o2  (d1 = 2*(8*h+o1p) + e1, d2 = 2*o2 + e2).
    xv = x[:].rearrange(
        "(s h o1p e1 o2 e2) f -> s h o1p e1 o2 e2 f",
        s=SLICES_PER_CORE, h=2, o1p=8, e1=2, o2=16, e2=2,
    )

    tiles = {}

    def src_ap(l: int, e1: int):
        s, h = l // 2, l % 2
        a = xv[s, h, :, e1, :, :, :]          # [o1p, o2, e2, f]
        return a.rearrange("o1p o2 e2 f -> o1p o2 (e2 f)")  # 8 KiB runs

    with tile.TileContext(nc) as tc:
        with (
            tc.tile_pool(name="inp", bufs=N_TILES) as inp,
            tc.tile_pool(name="m1p", bufs=4) as m1p,
            tc.tile_pool(name="m2p", bufs=4) as m2p,
            tc.tile_pool(name="obp", bufs=N_TILES) as obp,
        ):
            def dma_e0(l: int):
                t = inp.tile([128, 2048], F32, tag="t")
                tiles[l] = t
                nc.gpsimd.dma_start(t[:], src_ap(l, 0))

            def dma_e1(l: int):
                # CCE inline add: t += x[e1=1 rows]
                nc.gpsimd.dma_start(t_[l][:], src_ap(l, 1), accum_op=ADD)

            t_ = tiles

            def compute(l: int):
                t = tiles[l]
                # pool e2 (d2 pairs): [128, 2, 1024] -> [128, 1024]
                m1 = m1p.tile([128, 1024], F32, tag="m1")
                nc.vector.tensor_add(m1[:], t[:, 0:1024], t[:, 1024:2048])
                # pool d4 pairs: [128, 32d3, 16o4, 2e4] -> [128, 512]
                m1v = m1[:].rearrange("p (d3 o4 e4) -> p d3 o4 e4", d3=32, o4=16)
                m2 = m2p.tile([128, 512], F32, tag="m2")
                m2v = m2[:].rearrange("p (d3 o4) -> p d3 o4", d3=32)
                nc.vector.tensor_add(m2v, m1v[:, :, :, 0], m1v[:, :, :, 1])
                # pool d3 pairs: [128, 16o3, 2e3, 16o4] -> [128, 256]
                m2w = m2[:].rearrange("p (o3 e3 o4) -> p o3 e3 o4", o3=16, e3=2)
                m3 = m2p.tile([128, 256], F32, tag="m3")
                m3v = m3[:].rearrange("p (o3 o4) -> p o3 o4", o3=16)
                nc.vector.tensor_add(m3v, m2w[:, :, 0, :], m2w[:, :, 1, :])
                # scale by 1/16 (DVE tensor_scalar runs 2x for fp32 SBUF)
                ob = obp.tile([128, 256], F32, tag="ob")
                nc.vector.tensor_scalar_mul(ob[:], m3[:], 1.0 / 16.0)
                nc.sync.dma_start(y[128 * l : 128 * (l + 1), :], ob[:])

            # Interleave so the accumulating sub-DMA (which must wait for its
            # partner's last byte) trails by LAG transfers - the SWDGE queue
            # never stalls with an empty descriptor ring.
            for l in range(N_TILES):
                dma_e0(l)
                if l >= LAG:
                    dma_e1(l - LAG)
                    compute(l - LAG)
            for l in range(N_TILES - LAG, N_TILES):
                dma_e1(l)
                compute(l)

    nc.compile()
    return nc


_NC_CACHE: bass.Bass | None = None


def kernel(nd_tensor: np.ndarray, _trace: bool = False):
    global _NC_CACHE
    x = np.ascontiguousarray(np.asarray(nd_tensor, dtype=np.float32)).reshape(
        32, 1024, 1024
    )
    if _NC_CACHE is None:
        _NC_CACHE = build_nc()
    nc = _NC_CACHE

    in_maps = [
        {
            "x": np.ascontiguousarray(
                x[SLICES_PER_CORE * i : SLICES_PER_CORE * (i + 1)]
            ).reshape(ROWS, 1024)
        }
        for i in range(N_CORES)
    ]
    res = run_bass_kernel_spmd(
        nc, in_maps, core_ids=list(range(N_CORES)), trace=_trace
    )
    out = np.stack([res.results[i]["y"] for i in range(N_CORES)])  # [8,1024,256]
    out = out.reshape(2, 16, 16, 16, 16, 16).astype(np.float32)
    if _trace:
        kernel.last_results = res
    return out
